# revision 2
# baseline (speedup 1.0000x reference)
"""nn_Attention_68719476736027 — NATTEN-style 2D neighborhood attention block.

Strategy: 8-core SPMD Bass kernel on trn2. Sharding: batch (2) x H-quarters (4),
each core gets a 38-row edge-replicated slab (halo 3) and computes its 32 output
rows with interior-window semantics (no boundary clamping on device). The image
borders (rows/cols 0-2 and 125-127), where NATTEN's window clamping and the
reflect-padded depthwise conv differ from the interior formula, are computed on
the host (jax CPU jit on small crops, overlapped with the device round trip) and
pasted over the device result. I/O is bf16 to halve axon-tunnel transfer time.

Falls back to a pure CPU jax implementation of the full reference if the device
path fails for any reason.
"""

import threading
import numpy as np

DIM, HEADS, HD, K = 96, 4, 24, 7
HS, W, WP, OR_ = 38, 128, 134, 32
NPIX = HS * WP
SCALE = HD ** -0.5
B, H = 2, 128
NCORES = 8

_STATE = None
_STATE_LOCK = threading.Lock()
_CPU_FALLBACK = None


# ---------------------------------------------------------------------------
# Device kernel (per-core program, SPMD-shared)
# ---------------------------------------------------------------------------

def _build_bass():
    import concourse.bass as bass
    import concourse.mybir as mybir
    from concourse.ap import AP
    from concourse.tile import TileContext

    BF = mybir.dt.bfloat16
    F32 = mybir.dt.float32
    AL = mybir.AluOpType
    ACT = mybir.ActivationFunctionType

    nc = bass.Bass(debug=False)
    x = nc.declare_dram_parameter("x", [96, HS, W], BF, isOutput=False)
    wq = nc.declare_dram_parameter("wq", [97, 128], BF, isOutput=False)
    wk = nc.declare_dram_parameter("wk", [97, 128], BF, isOutput=False)
    wv = nc.declare_dram_parameter("wv", [97, 96], BF, isOutput=False)
    wp = nc.declare_dram_parameter("wp", [97, 96], BF, isOutput=False)
    cw = nc.declare_dram_parameter("cw", [96, 25], F32, isOutput=False)
    cb = nc.declare_dram_parameter("cb", [96, 1], F32, isOutput=False)
    eb = nc.declare_dram_parameter("eb", [1, 196], F32, isOutput=False)
    y = nc.declare_dram_parameter("y", [96, OR_, W], BF, isOutput=True)

    def _ap(t, off, dims):
        base = t[:]
        return AP(base.tensor, base.offset + off, dims)

    with TileContext(nc) as tc:
        with (
            tc.tile_pool(name="persist", bufs=1) as pp,
            tc.tile_pool(name="wpool", bufs=1) as wpl,
        ):
            xt = pp.tile([128, NPIX], BF, tag="xt")
            nc.vector.memset(xt[:], 0.0)
            nc.vector.memset(xt[96:97, :], 1.0)
            nc.sync.dma_start(
                out=_ap(xt, 3, [[NPIX, 96], [WP, HS], [1, W]]), in_=x.ap()
            )
            wqt = wpl.tile([97, 128], BF, tag="wq")
            wkt = wpl.tile([97, 128], BF, tag="wk")
            wvt = wpl.tile([97, 96], BF, tag="wv")
            wpt = wpl.tile([97, 96], BF, tag="wp")
            cwt = wpl.tile([96, 25], F32, tag="cw")
            cbt = wpl.tile([96, 1], F32, tag="cb")
            ebs = wpl.tile([1, 196], F32, tag="ebs")
            for t, src in ((wqt, wq), (wkt, wk), (wvt, wv), (wpt, wp),
                           (cwt, cw), (cbt, cb), (ebs, eb)):
                nc.sync.dma_start(out=t[:], in_=src.ap())
            ebt = pp.tile([128, 196], F32, tag="ebt")
            nc.gpsimd.partition_broadcast(ebt[:], ebs[:])

            idb = pp.tile([128, 128], BF, tag="idb")
            idf = pp.tile([128, 128], F32, tag="idf")
            for idt in (idb, idf):
                nc.gpsimd.memset(idt[:], 1.0)
                nc.gpsimd.affine_select(
                    out=idt[:], in_=idt[:], compare_op=AL.is_equal,
                    fill=0.0, base=0, pattern=[[-1, 128]], channel_multiplier=1,
                )

            qt = pp.tile([128, NPIX], BF, tag="qt")
            kt = pp.tile([128, NPIX], BF, tag="kt")
            vt = pp.tile([96, NPIX], BF, tag="vt")
            with tc.tile_pool(name="qkvps", bufs=4, space="PSUM") as qps:
                for dst, wt, m in ((qt, wqt, 128), (kt, wkt, 128), (vt, wvt, 96)):
                    c0 = 0
                    while c0 < NPIX:
                        n = min(512, NPIX - c0)
                        ps = qps.tile([m, 512], F32, tag="qkv_ps")
                        nc.tensor.matmul(
                            ps[:, 0:n], lhsT=wt[0:97, 0:m],
                            rhs=xt[0:97, c0:c0 + n], start=True, stop=True,
                        )
                        nc.any.tensor_copy(dst[:, c0:c0 + n], ps[:, 0:n])
                        c0 += n

            vtt = pp.tile([128, HS * 96], BF, tag="vtt")
            with tc.tile_pool(name="vtps", bufs=4, space="PSUM") as vps:
                for r in range(HS):
                    ps = vps.tile([128, 96], BF, tag="vt_ps")
                    nc.tensor.transpose(
                        ps[:], vt[0:96, r * WP + 3:r * WP + 131], idb[0:96, 0:96]
                    )
                    nc.any.tensor_copy(vtt[:, r * 96:(r + 1) * 96], ps[:])
            vts = []
            for s in range(K):
                tile = pp.tile([128, HS * 96], BF, tag=f"vts{s}")
                sh = s - 3
                q0, q1 = max(0, -sh), min(128, 128 - sh)
                if q0 > 0:
                    nc.vector.memset(tile[0:32, :], 0.0)
                if q1 < 128:
                    nc.vector.memset(tile[96:128, :], 0.0)
                nc.sync.dma_start(out=tile[q0:q1, :], in_=vtt[q0 + sh:q1 + sh, :])
                vts.append(tile)

            pt = pp.tile([128, OR_ * 196], BF, tag="pt")
            with (
                tc.tile_pool(name="sps", bufs=2, space="PSUM") as sps,
                tc.tile_pool(name="dens", bufs=3) as dpl,
            ):
                for o in range(OR_):
                    for h in range(HEADS):
                        sp = sps.tile([128, 1792], F32, tag="sp")
                        for a in range(K):
                            nc.tensor.matmul(
                                sp[:, a * 256:a * 256 + WP],
                                lhsT=qt[32 * h:32 * h + 32,
                                        (o + 3) * WP + 3:(o + 3) * WP + 131],
                                rhs=kt[32 * h:32 * h + 32,
                                       (o + a) * WP:(o + a + 1) * WP],
                                start=True, stop=True,
                                tile_position=(32 * h, 0),
                            )
                        nc.vector.tensor_copy(
                            _ap(pt, o * 196 + h * 49,
                                [[OR_ * 196, 128], [7, K], [1, K]]),
                            _ap(sp, 0, [[1793, 128], [256, K], [1, K]]),
                        )
                    sl = pt[:, o * 196:(o + 1) * 196]
                    nc.scalar.activation(sl, sl, ACT.Exp)
                    nc.vector.tensor_tensor(out=sl, in0=sl, in1=ebt[:], op=AL.mult)
                    den = dpl.tile([128, 4], F32, tag="den")
                    nc.vector.tensor_reduce(
                        den[:],
                        _ap(pt, o * 196, [[OR_ * 196, 128], [49, 4], [1, 49]]),
                        axis=mybir.AxisListType.X, op=AL.add,
                    )
                    rec = dpl.tile([128, 4], F32, tag="rec")
                    nc.vector.reciprocal(rec[:], den[:])
                    nc.vector.tensor_tensor(
                        out=_ap(pt, o * 196, [[OR_ * 196, 128], [49, 4], [1, 49]]),
                        in0=_ap(pt, o * 196, [[OR_ * 196, 128], [49, 4], [1, 49]]),
                        in1=_ap(rec, 0, [[rec[:].ap[0][0], 128], [1, 4], [0, 49]]),
                        op=AL.mult,
                    )

            acc = pp.tile([128, OR_ * 96], F32, tag="acc")
            with tc.tile_pool(name="avtmp", bufs=3) as tpl:
                first = True
                for a in range(K):
                    for s in range(K):
                        in0 = _ap(pt, a * 7 + s,
                                  [[OR_ * 196, 128], [196, OR_], [49, 4], [0, HD]])
                        in1 = _ap(vts[s], a * 96,
                                  [[HS * 96, 128], [96, OR_], [HD, 4], [1, HD]])
                        if first:
                            nc.vector.tensor_tensor(
                                out=_ap(acc, 0, [[OR_ * 96, 128], [96, OR_],
                                                 [HD, 4], [1, HD]]),
                                in0=in0, in1=in1, op=AL.mult)
                            first = False
                        else:
                            tmp = tpl.tile([128, OR_ * 96], BF, tag="avt")
                            nc.vector.tensor_tensor(
                                out=_ap(tmp, 0, [[OR_ * 96, 128], [96, OR_],
                                                 [HD, 4], [1, HD]]),
                                in0=in0, in1=in1, op=AL.mult)
                            nc.vector.tensor_tensor(
                                out=acc[:], in0=acc[:], in1=tmp[:], op=AL.add)

            cacc = pp.tile([96, OR_ * W], F32, tag="cacc")
            cview = _ap(cacc, 0, [[OR_ * W, 96], [W, OR_], [1, W]])
            tap = 0
            for dy in range(5):
                for dx in range(5):
                    in0 = _ap(vt, (1 + dy) * WP + 1 + dx,
                              [[NPIX, 96], [WP, OR_], [1, W]])
                    in1 = (cbt[:, 0:1].broadcast_to([96, OR_, W]) if tap == 0
                           else cview)
                    nc.vector.scalar_tensor_tensor(
                        out=cview, in0=in0, scalar=cwt[:, tap:tap + 1], in1=in1,
                        op0=AL.mult, op1=AL.add)
                    tap += 1

            comb = pp.tile([128, OR_ * W], BF, tag="comb")
            nc.vector.memset(comb[96:97, :], 1.0)
            with tc.tile_pool(name="tps2", bufs=4, space="PSUM") as tps2:
                for o in range(OR_):
                    ps = tps2.tile([96, 128], F32, tag="at_ps")
                    nc.tensor.transpose(
                        ps[:], acc[:, o * 96:(o + 1) * 96], idf[:, 0:128])
                    nc.vector.scalar_tensor_tensor(
                        out=comb[0:96, o * W:(o + 1) * W], in0=ps[:], scalar=1.0,
                        in1=_ap(cacc, o * W, [[OR_ * W, 96], [1, W]]),
                        op0=AL.mult, op1=AL.add)
            yout = pp.tile([96, OR_ * W], BF, tag="yout")
            with tc.tile_pool(name="pps", bufs=4, space="PSUM") as pps:
                c0 = 0
                while c0 < OR_ * W:
                    n = min(512, OR_ * W - c0)
                    ps = pps.tile([96, 512], F32, tag="proj_ps")
                    nc.tensor.matmul(
                        ps[:, 0:n], lhsT=wpt[0:97, 0:96],
                        rhs=comb[0:97, c0:c0 + n], start=True, stop=True)
                    nc.any.tensor_copy(yout[:, c0:c0 + n], ps[:, 0:n])
                    c0 += n
            nc.sync.dma_start(
                out=y.ap(),
                in_=_ap(yout, 0, [[OR_ * W, 96], [W, OR_], [1, W]]))
    return nc


# ---------------------------------------------------------------------------
# Cached PJRT runner (clone of bass2jax.run_bass_via_pjrt with a cached jit)
# ---------------------------------------------------------------------------

def _make_runner(nc, n_cores):
    import jax
    import numpy as _np
    from jax.sharding import Mesh, PartitionSpec
    from jax.experimental.shard_map import shard_map
    import concourse.mybir as mybir
    from concourse import bass2jax

    bass2jax.install_neuronx_cc_hook()
    in_names, out_names, out_avals, zero_shapes = [], [], [], []
    for alloc in nc.m.functions[0].allocations:
        if not isinstance(alloc, mybir.MemoryLocationSet):
            continue
        name = alloc.memorylocations[0].name
        if alloc.kind == "ExternalInput":
            in_names.append(name)
        elif alloc.kind == "ExternalOutput":
            shape = tuple(alloc.tensor_shape)
            dtype = mybir.dt.np(alloc.dtype)
            out_names.append(name)
            out_avals.append(jax.core.ShapedArray(shape, dtype))
            zero_shapes.append((shape, dtype))
    n_params = len(in_names)
    n_outs = len(out_avals)
    all_names = in_names + out_names
    donate = tuple(range(n_params, n_params + n_outs))

    def _body(*args):
        outs = bass2jax._bass_exec_p.bind(
            *args,
            out_avals=tuple(out_avals),
            in_names=tuple(all_names),
            out_names=tuple(out_names),
            lowering_input_output_aliases=(),
            sim_require_finite=True,
            sim_require_nnan=True,
            nc=nc,
        )
        return tuple(outs)

    devices = jax.devices()[:n_cores]
    assert len(devices) == n_cores
    mesh = Mesh(_np.asarray(devices), ("core",))
    in_specs = (PartitionSpec("core"),) * (n_params + n_outs)
    out_specs = (PartitionSpec("core"),) * n_outs
    sharded = jax.jit(
        shard_map(_body, mesh=mesh, in_specs=in_specs, out_specs=out_specs,
                  check_rep=False),
        donate_argnums=donate, keep_unused=True,
    )

    def run(in_maps):
        concat_in = [
            _np.concatenate([_np.asarray(in_maps[c][n]) for c in range(n_cores)],
                            axis=0)
            for n in in_names
        ]
        concat_zeros = [
            _np.zeros((n_cores * s[0], *s[1:]), d) for (s, d) in zero_shapes
        ]
        out_arrs = sharded(*concat_in, *concat_zeros)
        return out_arrs, out_names, [s for (s, _) in zero_shapes]

    return run


# ---------------------------------------------------------------------------
# Host-side: reference math for border crops (jax CPU)
# ---------------------------------------------------------------------------

def _make_border_fns():
    import jax
    import jax.numpy as jnp

    cpu = jax.devices("cpu")[0]

    def conv1x1(x, w, b):
        return jnp.einsum("bchw,oc->bohw", x, w[:, :, 0, 0]) + b[None, :, None, None]

    def dwconv5(x, w, b):
        xp = jnp.pad(x, ((0, 0), (0, 0), (2, 2), (2, 2)), mode="reflect")
        y = jax.lax.conv_general_dilated(
            xp, w, window_strides=(1, 1), padding="VALID",
            dimension_numbers=("NCHW", "OIHW", "NCHW"), feature_group_count=DIM)
        return y + b[None, :, None, None]

    def neighborhood_attn(qkv, rpb):
        Bn, C, Hn, Wn = qkv.shape
        t = qkv.reshape(Bn, 3, HEADS, HD, Hn, Wn).transpose(1, 0, 2, 4, 5, 3)
        q, k, v = t[0] * SCALE, t[1], t[2]
        half = K // 2
        I = jnp.clip(jnp.arange(Hn) - half, 0, Hn - K)[:, None] + jnp.arange(K)[None, :]
        J = jnp.clip(jnp.arange(Wn) - half, 0, Wn - K)[:, None] + jnp.arange(K)[None, :]
        knb = k[:, :, I[:, None, :, None], J[None, :, None, :], :]
        vnb = v[:, :, I[:, None, :, None], J[None, :, None, :], :]
        rbi = (K - 1) + I - jnp.arange(Hn)[:, None]
        rbj = (K - 1) + J - jnp.arange(Wn)[:, None]
        bias = rpb[:, rbi[:, None, :, None], rbj[None, :, None, :]]
        logits = jnp.einsum("bhijd,bhijkld->bhijkl", q, knb) + bias[None]
        attn = jax.nn.softmax(logits.reshape(Bn, HEADS, Hn, Wn, K * K), axis=-1)
        attn = attn.reshape(logits.shape)
        out = jnp.einsum("bhijkl,bhijkld->bhijd", attn, vnb)
        return out.transpose(0, 1, 4, 2, 3).reshape(Bn, C // 3, Hn, Wn)

    def ref(x, V_w, V_b, QK_w, QK_b, conv_w, conv_b, proj_w, proj_b, rpb):
        V = conv1x1(x, V_w, V_b)
        QKp = conv1x1(x, QK_w, QK_b)
        qkv = jnp.concatenate([QKp, V], axis=1)
        attn_out = neighborhood_attn(qkv, rpb)
        conv_out = dwconv5(V, conv_w, conv_b)
        return conv1x1(conv_out + attn_out, proj_w, proj_b)

    jref = jax.jit(ref, device=cpu)
    return jref


def _cpu_reference(inputs):
    global _CPU_FALLBACK
    if _CPU_FALLBACK is None:
        _CPU_FALLBACK = _make_border_fns()
    import jax
    args = [np.asarray(inputs[n], np.float32)
            for n in ("x", "V_w", "V_b", "QK_w", "QK_b", "conv_w", "conv_b",
                      "proj_w", "proj_b", "rpb")]
    return np.asarray(jax.block_until_ready(_CPU_FALLBACK(*args)), np.float32)


# ---------------------------------------------------------------------------
# Weight prep
# ---------------------------------------------------------------------------

def _prep_weights(inputs, bf16):
    Wqk = np.asarray(inputs["QK_w"], np.float32)[:, :, 0, 0]
    QKb = np.asarray(inputs["QK_b"], np.float32)
    wqf = np.zeros((97, 128), np.float32)
    wkf = np.zeros((97, 128), np.float32)
    for h in range(HEADS):
        wqf[:96, 32 * h:32 * h + HD] = Wqk[h * HD:(h + 1) * HD, :].T * SCALE
        wqf[96, 32 * h:32 * h + HD] = QKb[h * HD:(h + 1) * HD] * SCALE
        wkf[:96, 32 * h:32 * h + HD] = Wqk[96 + h * HD:96 + (h + 1) * HD, :].T
        wkf[96, 32 * h:32 * h + HD] = QKb[96 + h * HD:96 + (h + 1) * HD]
    Wv = np.asarray(inputs["V_w"], np.float32)[:, :, 0, 0]
    Wp = np.asarray(inputs["proj_w"], np.float32)[:, :, 0, 0]
    wvf = np.concatenate(
        [Wv.T, np.asarray(inputs["V_b"], np.float32)[None, :]], 0)
    wpf = np.concatenate(
        [Wp.T, np.asarray(inputs["proj_b"], np.float32)[None, :]], 0)
    return {
        "wq": wqf.astype(bf16), "wk": wkf.astype(bf16),
        "wv": wvf.astype(bf16), "wp": wpf.astype(bf16),
        "cw": np.ascontiguousarray(
            np.asarray(inputs["conv_w"], np.float32)[:, 0].reshape(96, 25)),
        "cb": np.asarray(inputs["conv_b"], np.float32).reshape(96, 1),
        "eb": np.ascontiguousarray(np.exp(
            np.asarray(inputs["rpb"], np.float32)[:, 3:10, 3:10]
        ).reshape(1, 196)),
    }


# ---------------------------------------------------------------------------
# Main entry
# ---------------------------------------------------------------------------

def _get_state():
    global _STATE
    with _STATE_LOCK:
        if _STATE is None:
            import ml_dtypes
            nc = _build_bass()
            runner = _make_runner(nc, NCORES)
            border = _make_border_fns()
            _STATE = {
                "runner": runner,
                "border": border,
                "bf16": ml_dtypes.bfloat16,
            }
        return _STATE


def _kernel_device(**inputs):
    st = _get_state()
    bf16 = st["bf16"]
    x = np.asarray(inputs["x"], np.float32)
    xb = x.astype(bf16)
    wts = _prep_weights(inputs, bf16)

    row_idx = [np.clip(np.arange(32 * qi - 3, 32 * qi + 35), 0, H - 1)
               for qi in range(4)]
    in_maps = []
    for c in range(NCORES):
        b, qi = divmod(c, 4)
        m = dict(wts)
        m["x"] = np.ascontiguousarray(xb[b, :, row_idx[qi], :].transpose(1, 0, 2))
        in_maps.append(m)

    # Kick the device (async dispatch) ...
    out_arrs, out_names, out_shapes = st["runner"](in_maps)

    # ... and overlap the border computation on the host.
    import jax
    fargs = [np.asarray(inputs[n], np.float32)
             for n in ("V_w", "V_b", "QK_w", "QK_b", "conv_w", "conv_b",
                       "proj_w", "proj_b", "rpb")]
    jref = st["border"]
    top = jref(x[:, :, 0:10, :], *fargs)
    bot = jref(x[:, :, H - 10:H, :], *fargs)
    lef = jref(x[:, :, :, 0:10], *fargs)
    rig = jref(x[:, :, :, H - 10:H], *fargs)
    top, bot, lef, rig = (np.asarray(jax.block_until_ready(a), np.float32)
                          for a in (top, bot, lef, rig))

    # Fetch + stitch device output.
    yc = np.asarray(out_arrs[0])  # (8*96, 32, 128) bf16
    yc = yc.reshape(NCORES, 96, OR_, W).astype(np.float32)
    out = np.empty((B, DIM, H, W), np.float32)
    for c in range(NCORES):
        b, qi = divmod(c, 4)
        out[b, :, 32 * qi:32 * qi + 32, :] = yc[c]
    out[:, :, 0:3, :] = top[:, :, 0:3, :]
    out[:, :, H - 3:H, :] = bot[:, :, 7:10, :]
    out[:, :, 3:H - 3, 0:3] = lef[:, :, 3:H - 3, 0:3]
    out[:, :, 3:H - 3, W - 3:W] = rig[:, :, 3:H - 3, 7:10]
    return out


def kernel(**inputs):
    try:
        return _kernel_device(**inputs)
    except Exception:
        import traceback
        traceback.print_exc()
        return _cpu_reference(inputs)
